# revision 14
# baseline (speedup 1.0000x reference)
"""Causal self-attention (B=2, T=2048, C=1024, H=16, D=64) on 8 trn2 NeuronCores.

Sharding: data-parallel over batch (2) x tensor-parallel over heads (16 -> 4
per core). Core c handles batch c//4 and head-quad c%4 (feature slice of 256).
Each core computes q/k/v projections for its 4 heads, causal attention, and a
partial output projection against its 256-column slice of Wo. The host sums
the 4 partials per batch (the TP all-reduce) and adds bo + Wo @ bv (the value
bias contributes exactly Wo @ bv per token since attention rows sum to 1).

All matmuls run as float32r (fp22) on the PE at full rate. Scores are computed
transposed (S^T[t, q]) so the scalar engine's exp writes P^T directly in the
layout the P@V matmul consumes; softmax runs without max-subtraction (logits
are bounded by |q||k|/8 <= 8) and the denominator comes from an appended
ones-column in the V stationary operand.
"""

import numpy as np

B = 2
T = 2048
C = 1024
NH = 16
D = 64
HEADS_PER_CORE = 4
FSLICE = HEADS_PER_CORE * D  # 256 features per core
SCALE = 0.125  # 1/sqrt(64)
N_CORES = 8

TOKB = T // 128  # 16 token blocks
KCH = C // 128  # 8 contraction chunks
QCH = T // 512  # 4 q chunks


def _build_nc():
    import concourse.bacc as bacc
    import concourse.mybir as mybir
    import concourse.tile as tile
    from concourse.masks import make_identity
    from contextlib import ExitStack

    F32 = mybir.dt.float32
    F32R = mybir.dt.float32r

    nc = bacc.Bacc()
    x_d = nc.dram_tensor("x", [T, C], F32, kind="ExternalInput")
    wq_d = nc.dram_tensor("wq", [FSLICE, C], F32, kind="ExternalInput")
    wk_d = nc.dram_tensor("wk", [FSLICE, C], F32, kind="ExternalInput")
    wv_d = nc.dram_tensor("wv", [FSLICE, C], F32, kind="ExternalInput")
    wo_d = nc.dram_tensor("wo", [C, FSLICE], F32, kind="ExternalInput")
    bq_d = nc.dram_tensor("bq", [1, FSLICE], F32, kind="ExternalInput")
    bk_d = nc.dram_tensor("bk", [1, FSLICE], F32, kind="ExternalInput")
    out_d = nc.dram_tensor("out", [T, C], F32, kind="ExternalOutput")

    with tile.TileContext(nc) as tc, ExitStack() as top:
        # ---- persistent SBUF ----
        perm = top.enter_context(tc.tile_pool(name="perm", bufs=1))
        ident = perm.tile([128, 128], F32)
        make_identity(nc, ident)
        ones_f32 = perm.tile([1, 512], F32)
        nc.vector.memset(ones_f32, 1.0)
        ones512 = perm.tile([1, 512], F32R)
        nc.vector.tensor_copy(ones512, ones_f32)
        ones64 = perm.tile([1, 64], F32R)
        nc.vector.tensor_copy(ones64, ones_f32[:, 0:64])
        bq_sb = perm.tile([1, FSLICE], F32R)
        bk_sb = perm.tile([1, FSLICE], F32R)
        nc.sync.dma_start(out=bq_sb, in_=bq_d[:, :].bitcast(F32R))
        nc.sync.dma_start(out=bk_sb, in_=bk_d[:, :].bitcast(F32R))

        wqT = perm.tile([128, KCH, FSLICE], F32R)  # [c, kc, feat]
        wkT = perm.tile([128, KCH, FSLICE], F32R)
        wvT = perm.tile([128, KCH, FSLICE], F32R)
        woT = perm.tile([128, 2, C], F32R)  # [feat, fc, out]
        qT = [perm.tile([128, T], F32R, name=f"qT{p}") for p in range(2)]
        kT = [perm.tile([128, T], F32R, name=f"kT{p}") for p in range(2)]
        # v with an appended ones column per head: [tok, head, 65]
        v_sb = perm.tile([128, TOKB, HEADS_PER_CORE, D + 1], F32R)
        oT = [perm.tile([128, T], F32R, name=f"oT{p}") for p in range(2)]

        ones_col = perm.tile([128, TOKB * HEADS_PER_CORE], F32)
        nc.vector.memset(ones_col, 1.0)
        nc.vector.tensor_copy(
            v_sb[:, :, :, D : D + 1].rearrange("p a b c -> p (a b c)"), ones_col
        )

        # ---- phase A/B: load + transpose x and weights ----
        with ExitStack() as ph:
            xT_pool = ph.enter_context(tc.tile_pool(name="xT", bufs=1))
            xT = xT_pool.tile([128, KCH, T], F32R)  # [c, kc, tok]
            raw = ph.enter_context(tc.tile_pool(name="raw", bufs=3))
            tps = ph.enter_context(tc.tile_pool(name="tps", bufs=2, space="PSUM"))

            def transpose_to(dst_ap, src_ap):
                tp = tps.tile([128, 128], F32, name="tp")
                nc.tensor.transpose(tp, src_ap.bitcast(F32), ident)
                nc.vector.tensor_copy(dst_ap, tp)

            for tb in range(TOKB):
                x_raw = raw.tile([128, C], F32R, name="x_raw")
                nc.sync.dma_start(
                    out=x_raw, in_=x_d[tb * 128 : (tb + 1) * 128, :].bitcast(F32R)
                )
                for kc in range(KCH):
                    transpose_to(
                        xT[:, kc, tb * 128 : (tb + 1) * 128],
                        x_raw[:, kc * 128 : (kc + 1) * 128],
                    )
            for w_d, wT in ((wq_d, wqT), (wk_d, wkT), (wv_d, wvT)):
                for g in range(2):
                    w_raw = raw.tile([128, C], F32R, name="w_raw")
                    nc.sync.dma_start(
                        out=w_raw,
                        in_=w_d[g * 128 : (g + 1) * 128, :].bitcast(F32R),
                    )
                    for kc in range(KCH):
                        transpose_to(
                            wT[:, kc, g * 128 : (g + 1) * 128],
                            w_raw[:, kc * 128 : (kc + 1) * 128],
                        )
            for ob in range(KCH):
                wo_raw = raw.tile([128, FSLICE], F32R, name="wo_raw")
                nc.sync.dma_start(
                    out=wo_raw,
                    in_=wo_d[ob * 128 : (ob + 1) * 128, :].bitcast(F32R),
                )
                for fc in range(2):
                    transpose_to(
                        woT[:, fc, ob * 128 : (ob + 1) * 128],
                        wo_raw[:, fc * 128 : (fc + 1) * 128],
                    )

            # ---- phase C: projections (share psum pools with transposes) ----
            qkps = ph.enter_context(tc.tile_pool(name="qkps", bufs=4, space="PSUM"))
            vpsp = ph.enter_context(tc.tile_pool(name="vpsp", bufs=2, space="PSUM"))
            for wT, bias_sb, dstT in ((wqT, bq_sb, qT), (wkT, bk_sb, kT)):
                for p in range(2):
                    psums = []
                    for qc in range(QCH):
                        ps = qkps.tile([128, 512], F32, name="qk_ps")
                        psums.append(ps)
                        # bias via rank-1 matmul: bias[f] * ones[tok]
                        nc.tensor.matmul(
                            ps,
                            bias_sb[:, p * 128 : (p + 1) * 128],
                            ones512,
                            start=True,
                            stop=False,
                        )
                    for kc in range(KCH):
                        for qc in range(QCH):
                            nc.tensor.matmul(
                                psums[qc],
                                wT[:, kc, p * 128 : (p + 1) * 128],
                                xT[:, kc, qc * 512 : (qc + 1) * 512],
                                start=False,
                                stop=(kc == KCH - 1),
                            )
                    for qc in range(QCH):
                        nc.vector.tensor_copy(
                            dstT[p][:, qc * 512 : (qc + 1) * 512],
                            psums[qc].bitcast(F32R),
                        )
            # v projection: natural [tok, feat] layout
            for tb in range(TOKB):
                vps = vpsp.tile([128, FSLICE], F32, name="v_ps")
                for kc in range(KCH):
                    nc.tensor.matmul(
                        vps,
                        xT[:, kc, tb * 128 : (tb + 1) * 128],
                        wvT[:, kc, :],
                        start=(kc == 0),
                        stop=(kc == KCH - 1),
                    )
                nc.vector.tensor_copy(
                    v_sb[:, tb, :, 0:D],
                    vps.rearrange("p (h d) -> p h d", h=HEADS_PER_CORE).bitcast(F32R),
                )

        # ---- phase D: attention ----
        import concourse.mybir as mb

        with ExitStack() as ph:
            scps = ph.enter_context(tc.tile_pool(name="scps", bufs=2, space="PSUM"))
            pvps = ph.enter_context(tc.tile_pool(name="pvps", bufs=1, space="PSUM"))
            bcps = ph.enter_context(tc.tile_pool(name="bcps", bufs=2, space="PSUM"))
            ptp = ph.enter_context(tc.tile_pool(name="ptp", bufs=4))
            rcp = ph.enter_context(tc.tile_pool(name="rcp", bufs=4))

            for p in range(2):
                for qc in range(QCH):
                    ntb = 4 * qc + 4
                    pv0 = pvps.tile([65, 512], F32, name="pv0")
                    pv1 = pvps.tile([65, 512], F32, name="pv1")
                    pvs = (pv0, pv1)
                    for tb in range(ntb):
                        sc = scps.tile([128, 1024], F32, name="sc")
                        for h2 in range(2):
                            nc.tensor.matmul(
                                sc[:, h2 * 512 : (h2 + 1) * 512],
                                kT[p][
                                    h2 * 64 : (h2 + 1) * 64,
                                    tb * 128 : (tb + 1) * 128,
                                ],
                                qT[p][
                                    h2 * 64 : (h2 + 1) * 64,
                                    qc * 512 : (qc + 1) * 512,
                                ],
                                start=True,
                                stop=True,
                            )
                        pt = ptp.tile([128, 2, 512], F32R, name="pt")
                        nc.scalar.activation(
                            pt,
                            sc.rearrange("p (h q) -> p h q", h=2),
                            mb.ActivationFunctionType.Exp,
                            scale=SCALE,
                        )
                        if tb >= 4 * qc:
                            # causal: keep where (qc*512 + j) - (tb*128 + part) >= 0
                            nc.gpsimd.affine_select(
                                out=pt,
                                in_=pt,
                                compare_op=mb.AluOpType.is_ge,
                                fill=0.0,
                                base=qc * 512 - tb * 128,
                                channel_multiplier=-1,
                                pattern=[[0, 2], [1, 512]],
                            )
                        for h2 in range(2):
                            nc.tensor.matmul(
                                pvs[h2],
                                v_sb[:, tb, 2 * p + h2, 0 : D + 1],
                                pt[:, h2, :],
                                start=(tb == 0),
                                stop=(tb == ntb - 1),
                            )
                    for h2 in range(2):
                        recip = rcp.tile([1, 512], F32R, name="recip")
                        with nc.allow_low_precision(reason="fp22 softmax denom"):
                            nc.vector.reciprocal(recip, pvs[h2][64:65, :])
                        bc = bcps.tile([64, 512], F32, name="bc")
                        nc.tensor.matmul(bc, ones64, recip, start=True, stop=True)
                        bc_sb = rcp.tile([64, 512], F32, name="bc_sb")
                        nc.scalar.copy(bc_sb, bc)
                        nc.vector.tensor_tensor(
                            out=oT[p][
                                h2 * 64 : (h2 + 1) * 64, qc * 512 : (qc + 1) * 512
                            ],
                            in0=pvs[h2][0:64, :],
                            in1=bc_sb,
                            op=mb.AluOpType.mult,
                        )

        # ---- phase E: output projection ----
        with ExitStack() as ph:
            pjps = ph.enter_context(tc.tile_pool(name="pjps", bufs=4, space="PSUM"))
            outp = ph.enter_context(tc.tile_pool(name="outp", bufs=3))
            for tb in range(TOKB):
                ostage = outp.tile([128, C], F32, name="ostage")
                for oc in range(2):
                    pj = pjps.tile([128, 512], F32, name="pj")
                    for p in range(2):
                        nc.tensor.matmul(
                            pj,
                            oT[p][:, tb * 128 : (tb + 1) * 128],
                            woT[:, p, oc * 512 : (oc + 1) * 512],
                            start=(p == 0),
                            stop=(p == 1),
                        )
                    nc.vector.tensor_copy(ostage[:, oc * 512 : (oc + 1) * 512], pj)
                nc.sync.dma_start(
                    out=out_d[tb * 128 : (tb + 1) * 128, :], in_=ostage
                )

    nc.compile()
    return nc


_NC_CACHE = None


def _get_nc():
    global _NC_CACHE
    if _NC_CACHE is None:
        _NC_CACHE = _build_nc()
    return _NC_CACHE


def make_in_maps(x, Wq, bq, Wk, bk, Wv, bv, Wo, bo):
    x = np.asarray(x, dtype=np.float32)
    in_maps = []
    for c in range(N_CORES):
        b, p4 = divmod(c, 4)
        fs = slice(p4 * FSLICE, (p4 + 1) * FSLICE)
        in_maps.append(
            {
                "x": np.ascontiguousarray(x[b]),
                "wq": np.ascontiguousarray(np.asarray(Wq)[fs, :]),
                "wk": np.ascontiguousarray(np.asarray(Wk)[fs, :]),
                "wv": np.ascontiguousarray(np.asarray(Wv)[fs, :]),
                "wo": np.ascontiguousarray(np.asarray(Wo)[:, fs]),
                "bq": np.ascontiguousarray(np.asarray(bq)[fs].reshape(1, FSLICE)),
                "bk": np.ascontiguousarray(np.asarray(bk)[fs].reshape(1, FSLICE)),
            }
        )
    return in_maps


def combine_outputs(outs, Wo, bv, bo):
    """outs: list of 8 [T, C] partials. Host-side TP all-reduce + biases."""
    const = np.asarray(bo, dtype=np.float32) + np.asarray(Wo, dtype=np.float32) @ np.asarray(
        bv, dtype=np.float32
    )
    full = np.stack(outs).reshape(B, 4, T, C).sum(axis=1, dtype=np.float32)
    return (full + const[None, None, :]).astype(np.float32)


def kernel(x, Wq, bq, Wk, bk, Wv, bv, Wo, bo):
    from concourse.bass_utils import run_bass_kernel_spmd

    nc = _get_nc()
    in_maps = make_in_maps(x, Wq, bq, Wk, bk, Wv, bv, Wo, bo)
    res = run_bass_kernel_spmd(nc, in_maps, core_ids=list(range(N_CORES)))
    outs = [res.results[c]["out"] for c in range(N_CORES)]
    return combine_outputs(outs, Wo, bv, bo)
